# revision 1
# baseline (speedup 1.0000x reference)
"""Trainium2 Bass kernel for nn_LinearEffects (iterated conv1d with
per-sample mean renormalization).

Algorithm notes
---------------
reference: m_{t} = relu(conv1d(m_{t-1}, mu, pad=10) + x0) * adj_t with
adj_t = target_mean / (1e-5 + mean(relu_out)), m_0 = x0 = m0^T.

Device-side trick: since conv is linear, the per-sample scalar adj is
folded into the 21 (64,64) weight taps each iteration instead of
rescaling the 4 MiB activation: the SBUF activation always holds the
*unscaled* relu output, and the last iteration's adj is applied on the
host after gather.

Layout trick: C=64 would waste half of the 128x128 PE array.  We store
the activation interleaved: partitions 0:64 = even L positions,
64:128 = odd L positions, i.e. an (128, L/2) buffer.  The conv then
becomes 11 matmuls of K=128 (2 taps x 64ch), M=128 (2 output parities
x 64 ch), N=512 per output chunk at ~95% PE utilization, with weights
prepared host-side as 11 (128,128) stationary tiles:
  ST[r][(h,ci),(p,co)] = mu[co,ci, 2r+h-p+10]  (zero if out of range)
The psum chunk (128,512) is exactly the interleaved layout of 1024
consecutive output positions, so the +x0 / relu / in-place update all
stay in the interleaved layout on full 128 partitions.

Sharding: pure data parallel, 4 of 32 batch samples per NeuronCore.
"""

import numpy as np
from contextlib import ExitStack

import concourse.bass as bass
import concourse.bacc as bacc
import concourse.tile as tile
import concourse.bass_isa as bass_isa
from concourse import mybir
from concourse.tile import add_dep_helper
from concourse.bass_utils import run_bass_kernel_spmd

N_CORES = 8
B_FULL, L_FULL, C, W = 32, 16384, 64, 21
HAL = 5          # halo columns each side of the interleaved buffer
NTAP = 11        # 21 taps -> 11 paired stationary tiles
CHUNK = 512      # matmul free dim (one psum bank)
GRP = 4          # chunks per eviction group (psum tile = GRP banks)

f32 = mybir.dt.float32
f32r = mybir.dt.float32r
ALU = mybir.AluOpType
ACTF = mybir.ActivationFunctionType


def _build(S, Lh, nit):
    """Build the per-core Bass program: S samples, interleaved width Lh
    (=L/2), nit fixed-point iterations."""
    Wd = HAL + Lh + HAL
    NCH = Lh // CHUNK
    GRPL = min(GRP, NCH)
    bconst = float(C * 2 * Lh) * 1e-5

    nc = bacc.Bacc("TRN2", target_bir_lowering=False, debug=False)
    x0e = nc.dram_tensor("x0e", [S, 128, Wd], f32r, kind="ExternalInput")
    stat = nc.dram_tensor("stat", [NTAP, 128, 128], f32r, kind="ExternalInput")
    amat = nc.dram_tensor("amat", [S, 128, 1], f32, kind="ExternalInput")
    out = nc.dram_tensor("out", [S, 128, Lh], f32r, kind="ExternalOutput")

    dmas = []

    def dma(eng, o, i):
        inst = eng.dma_start(out=o, in_=i)
        dmas.append(inst.ins)
        return inst

    with tile.TileContext(nc) as tc, ExitStack() as ctx, \
            nc.allow_low_precision(reason="fp32r (fp22-mantissa) matmul path; "
                                   "accumulation stays fp32 in PSUM"):
        pool = lambda name, bufs, **kw: ctx.enter_context(
            tc.tile_pool(name=name, bufs=bufs, **kw))
        stb_pool = pool("stbase", 1)
        x0_pool = pool("x0", 2)
        b_pool = pool("bbuf", 2)
        stw_pool = pool("stw", 3)
        am_pool = pool("am", 2)
        sums_pool = pool("sums", 3)
        tmp_pool = pool("tmp", 4)
        small_pool = pool("small", 6)
        psum_pool = pool("psum", 2, space="PSUM")

        stb = stb_pool.tile([128, NTAP * 128], f32r)
        for i in range(NTAP):
            dma(nc.sync, stb[:, i * 128:(i + 1) * 128], stat[i])

        # per-sample persistent state
        st_cur = [None] * S   # stationary tiles scaled by adj_{t-1}
        Xt = [None] * S
        Bt = [None] * S
        AMt = [None] * S

        def load_sample(s):
            Xt[s] = x0_pool.tile([128, Wd], f32r, name="x0t", tag="x0t")
            dma(nc.sync, Xt[s][:], x0e[s])
            Bt[s] = b_pool.tile([128, Wd], f32r, name="bbt", tag="bbt")
            dma(nc.sync, Bt[s][:], x0e[s])
            AMt[s] = am_pool.tile([128, 1], f32, name="amt", tag="amt")
            dma(nc.sync, AMt[s][:], amat[s])
            st_cur[s] = stb  # iteration 1 has adj = 1

        def iteration(s, t):
            X0, Bu, stw = Xt[s], Bt[s], st_cur[s]
            last = t == nit
            NG = NCH // GRPL  # eviction groups of GRPL chunks
            sums = None if last else sums_pool.tile([128, NG], f32)

            def conv(g):
                # one (128, GRP*CHUNK) psum tile = GRP banks; each chunk's 11
                # accumulating matmuls target its own 512-wide bank slice
                ps = psum_pool.tile([128, GRPL * CHUNK], f32)
                for k in range(GRPL):
                    c0 = HAL + CHUNK * (g * GRPL + k)
                    for ri in range(NTAP):
                        r = ri - 5
                        nc.tensor.matmul(
                            ps[:, k * CHUNK:(k + 1) * CHUNK],
                            stw[:, ri * 128:(ri + 1) * 128],
                            Bu[:, c0 + r:c0 + r + CHUNK],
                            start=(ri == 0), stop=(ri == NTAP - 1))
                return ps

            def evict(g, ps):
                GW = GRPL * CHUNK
                c0 = HAL + GW * g
                tmp = tmp_pool.tile([128, GW], f32)
                nc.vector.tensor_tensor(
                    tmp[:], ps[:], X0[:, c0:c0 + GW], ALU.add)
                if last:
                    nc.scalar.activation(Bu[:, c0:c0 + GW], tmp[:], ACTF.Relu)
                    dma(nc.sync, out[s, :, GW * g:GW * (g + 1)],
                        Bu[:, c0:c0 + GW])
                else:
                    nc.scalar.activation(Bu[:, c0:c0 + GW], tmp[:],
                                         ACTF.Relu, accum_out=sums[:, g:g + 1])

            prev = None
            for g in range(NG):
                ps = conv(g)
                if prev is not None:
                    evict(g - 1, prev)
                prev = ps
            evict(NG - 1, prev)

            if not last:
                # adj = A_s / (bconst + total_sum); fold into stationary tiles
                part = small_pool.tile([128, 1], f32)
                nc.vector.tensor_reduce(part[:], sums[:], mybir.AxisListType.X,
                                        ALU.add)
                stot = small_pool.tile([128, 1], f32)
                nc.gpsimd.partition_all_reduce(stot[:], part[:], 128,
                                               bass_isa.ReduceOp.add)
                sb = small_pool.tile([128, 1], f32)
                nc.vector.tensor_scalar_add(sb[:], stot[:], bconst)
                rec1 = small_pool.tile([128, 1], f32)
                nc.vector.reciprocal(rec1[:], sb[:])
                adjt = small_pool.tile([128, 1], f32)
                nc.vector.tensor_tensor(adjt[:], rec1[:], AMt[s][:], ALU.mult)
                stw2 = stw_pool.tile([128, NTAP * 128], f32r)
                nc.vector.tensor_scalar_mul(stw2[:], stb[:], adjt[:])
                st_cur[s] = stw2

        # two resident sample slots, pairs processed in lockstep so one
        # sample's conv hides the other's iteration-boundary latency
        for p0 in range(0, S, 2):
            pair = [p0] if p0 + 1 >= S else [p0, p0 + 1]
            for s in pair:
                load_sample(s)
            for t in range(1, nit + 1):
                for s in pair:
                    iteration(s, t)

    # bacc's pipeline splits multi-waits (this walrus accepts one sync
    # wait per instruction) via event semaphores
    nc.compile()
    return nc


def _prep(m0, mu, n_cores):
    Bn, L, Cn = m0.shape
    Lh = L // 2
    Wd = HAL + Lh + HAL
    x0 = np.ascontiguousarray(m0.transpose(0, 2, 1))          # (B, C, L)
    tmean = x0.reshape(Bn, -1).mean(1, dtype=np.float32)
    A = tmean.astype(np.float64) * (Cn * L)

    E = np.zeros((Bn, 128, Wd), np.float32)
    E[:, :64, HAL:HAL + Lh] = x0[:, :, 0::2]
    E[:, 64:, HAL:HAL + Lh] = x0[:, :, 1::2]

    ST = np.zeros((NTAP, 128, 128), np.float32)
    for ri in range(NTAP):
        r = ri - 5
        for h in (0, 1):
            for p in (0, 1):
                w = 2 * r + h - p + 10
                if 0 <= w <= W - 1:
                    ST[ri, h * 64:(h + 1) * 64, p * 64:(p + 1) * 64] = \
                        mu[:, :, w].T
    AM = np.broadcast_to(
        A.astype(np.float32)[:, None, None], (Bn, 128, 1)).copy()
    return E, ST, AM, tmean


def kernel(m0, mu, num_iterations):
    m0 = np.asarray(m0, dtype=np.float32)
    mu = np.asarray(mu, dtype=np.float32)
    nit = int(num_iterations)
    if nit <= 0:
        return m0.copy()

    Bn, L, Cn = m0.shape
    S = Bn // N_CORES
    Lh = L // 2
    E, ST, AM, tmean = _prep(m0, mu, N_CORES)

    nc = _build(S, Lh, nit)
    in_maps = [
        {"x0e": E[k * S:(k + 1) * S],
         "stat": ST,
         "amat": AM[k * S:(k + 1) * S]}
        for k in range(N_CORES)
    ]
    res = run_bass_kernel_spmd(nc, in_maps, list(range(N_CORES)))

    outs = np.concatenate([res.results[k]["out"] for k in range(N_CORES)], 0)
    # final adj (the reference's last in-loop rescale) applied host-side
    ssum = outs.reshape(Bn, -1).sum(1, dtype=np.float64)
    adj = tmean.astype(np.float64) / (1e-5 + ssum / (Cn * L))

    m_cl = np.empty((Bn, Cn, L), np.float32)
    m_cl[:, :, 0::2] = outs[:, :64, :]
    m_cl[:, :, 1::2] = outs[:, 64:, :]
    m_cl *= adj[:, None, None].astype(np.float32)
    return np.ascontiguousarray(m_cl.transpose(0, 2, 1))

